# revision 26
# baseline (speedup 1.0000x reference)
"""Trainium2 Bass kernel for nn_AdjustNet (sparse_attention).

Data-parallel over batch B=32 across 8 NeuronCores (4 batch elements per core).
Per batch element (all on-device):
  sample_layer : gelu(ln(query^T @ w1 + b1)) @ w2 + b2      (over sequence axis)
  adjust_layer : adj = gelu(ln(q @ w3 + b3)) @ w4 + b4 ; tanh*0.02
  pos = tanh(offset + adj + ref)
  out = causal bilinear gather of x rows (dma_gather, bf16) + lerp.

Self-contained: hardcodes shapes; host-side work is only sharding/assembly and
constant-table generation.
"""
import os
import numpy as np

B, L, C, N = 32, 512, 256, 16
NCORES = 8
BPC = B // NCORES          # batch elements per core
P = 128
KQ = L // P                # 4 contraction chunks over sequence (mm1)
LC = L // P                # 4 l-chunks
J = L // 4                 # 128 bottleneck dim of sample_layer
EPS = 1e-12
ELEM = 2 * C               # gather element: two consecutive x rows
NIDX = P * N               # gather indices per (batch, l-chunk)
RSQRT_MAGIC = 0x5F3759DF

_CACHE = {}


def _build():
    import concourse.bacc as bacc
    import concourse.bass as bass
    import concourse.mybir as mybir
    import concourse.tile as tile
    from concourse.library_config import mlp

    fp32 = mybir.dt.float32
    bf16 = mybir.dt.bfloat16
    i16 = mybir.dt.int16
    i32 = mybir.dt.int32
    AF = mybir.ActivationFunctionType
    OP = mybir.AluOpType

    nc = bacc.Bacc("TRN2", target_bir_lowering=False, debug=False)

    # ---------------- per-core DRAM I/O ----------------
    query_d = nc.dram_tensor("query", [BPC, L, C], fp32, kind="ExternalInput")
    off_d = nc.dram_tensor("offset", [BPC, L, 2 * N], fp32, kind="ExternalInput")
    w1_d = nc.dram_tensor("w1", [L, J], fp32, kind="ExternalInput")
    b1_d = nc.dram_tensor("b1", [J], fp32, kind="ExternalInput")
    g1_d = nc.dram_tensor("g1", [J], fp32, kind="ExternalInput")
    bt1_d = nc.dram_tensor("bt1", [J], fp32, kind="ExternalInput")
    w2_d = nc.dram_tensor("w2", [J, L], fp32, kind="ExternalInput")
    b2_d = nc.dram_tensor("b2", [L], fp32, kind="ExternalInput")
    w3_d = nc.dram_tensor("w3", [C, C], fp32, kind="ExternalInput")
    b3_d = nc.dram_tensor("b3", [C], fp32, kind="ExternalInput")
    g3_d = nc.dram_tensor("g3", [C], fp32, kind="ExternalInput")
    bt3_d = nc.dram_tensor("bt3", [C], fp32, kind="ExternalInput")
    w4_d = nc.dram_tensor("w4", [C, 2 * N], fp32, kind="ExternalInput")
    b4_d = nc.dram_tensor("b4", [2 * N], fp32, kind="ExternalInput")
    ref_d = nc.dram_tensor("cst_ref", [L, 2 * N], fp32, kind="ExternalInput")
    lpos_d = nc.dram_tensor("cst_lpos", [P, 2 * LC * N], fp32, kind="ExternalInput")
    ident_d = nc.dram_tensor("cst_ident", [P, P], fp32, kind="ExternalInput")

    out_d = nc.dram_tensor("out", [BPC * L, N * C], fp32, kind="ExternalOutput")
    pos_d = nc.dram_tensor("pos", [BPC * L, 2 * N], fp32, kind="ExternalOutput")

    x16_d = nc.dram_tensor("x16", [BPC * L, C], bf16, kind="ExternalInput")

    def pbcast(dram_1d, n):
        # replicate a [n] DRAM vector across 128 partitions
        return bass.AP(tensor=dram_1d, offset=0, ap=[[0, P], [1, n]])

    def fbcast(ap2d, inner):
        # broadcast a [P, F] AP along a new innermost dim of size `inner`
        return bass.AP(tensor=ap2d.tensor, offset=ap2d.offset, ap=list(ap2d.ap) + [[0, inner]])

    # overlapping 2-row window of bf16 x: idx i -> 512 consecutive bf16 from row i
    x_win = bass.AP(tensor=x16_d, offset=0, ap=[[C, BPC * L - 1], [1, ELEM]])

    with tile.TileContext(nc) as tc:
        import contextlib
        with contextlib.ExitStack() as ctx:
            cst = ctx.enter_context(tc.tile_pool(name="cst", bufs=1))
            qpool = ctx.enter_context(tc.tile_pool(name="qpool", bufs=2))
            apool = ctx.enter_context(tc.tile_pool(name="apool", bufs=2))
            h2pool = ctx.enter_context(tc.tile_pool(name="h2pool", bufs=2))
            bpool = ctx.enter_context(tc.tile_pool(name="bpool", bufs=2))
            gpool = ctx.enter_context(tc.tile_pool(name="gpool", bufs=5))
            lpool = ctx.enter_context(tc.tile_pool(name="lpool", bufs=2))
            drampool = ctx.enter_context(tc.tile_pool(name="drampool", bufs=3, space="DRAM"))
            ps_a = ctx.enter_context(tc.tile_pool(name="ps_a", bufs=2, space="PSUM"))
            ps_t = ctx.enter_context(tc.tile_pool(name="ps_t", bufs=2, space="PSUM"))
            ps_h2 = ctx.enter_context(tc.tile_pool(name="ps_h2", bufs=1, space="PSUM"))
            ps_h3 = ctx.enter_context(tc.tile_pool(name="ps_h3", bufs=2, space="PSUM"))
            ps_adj = ctx.enter_context(tc.tile_pool(name="ps_adj", bufs=1, space="PSUM"))

            nc.gpsimd.load_library(mlp)

            # ---------------- constants into SBUF ----------------
            w1_sb = cst.tile([P, KQ, J], fp32)
            nc.sync.dma_start(out=w1_sb[:], in_=w1_d.ap().rearrange("(k p) j -> p k j", p=P))
            w2_sb = cst.tile([P, L], fp32)
            nc.sync.dma_start(out=w2_sb[:], in_=w2_d.ap())
            w3_sb = cst.tile([P, 2, C], fp32)
            nc.sync.dma_start(out=w3_sb[:], in_=w3_d.ap().rearrange("(k p) c -> p k c", p=P))
            w4_sb = cst.tile([P, 2, 2 * N], fp32)
            nc.sync.dma_start(out=w4_sb[:], in_=w4_d.ap().rearrange("(k p) m -> p k m", p=P))
            b1_bc = cst.tile([P, J], fp32)
            nc.sync.dma_start(out=b1_bc[:], in_=pbcast(b1_d, J))
            g1_bc = cst.tile([P, J], fp32)
            nc.sync.dma_start(out=g1_bc[:], in_=pbcast(g1_d, J))
            bt1_bc = cst.tile([P, J], fp32)
            nc.sync.dma_start(out=bt1_bc[:], in_=pbcast(bt1_d, J))
            b2_bc = cst.tile([P, L], fp32)
            nc.sync.dma_start(out=b2_bc[:], in_=pbcast(b2_d, L))
            b3_bc = cst.tile([P, C], fp32)
            nc.sync.dma_start(out=b3_bc[:], in_=pbcast(b3_d, C))
            g3_bc = cst.tile([P, C], fp32)
            nc.sync.dma_start(out=g3_bc[:], in_=pbcast(g3_d, C))
            bt3_bc = cst.tile([P, C], fp32)
            nc.sync.dma_start(out=bt3_bc[:], in_=pbcast(bt3_d, C))
            b4_bc = cst.tile([P, 2 * N], fp32)
            nc.sync.dma_start(out=b4_bc[:], in_=pbcast(b4_d, 2 * N))
            ref_sb = cst.tile([P, LC, 2 * N], fp32)
            nc.sync.dma_start(out=ref_sb[:], in_=ref_d.ap().rearrange("(lc p) f -> p lc f", p=P))
            lpos_sb = cst.tile([P, 2 * LC * N], fp32)
            nc.sync.dma_start(out=lpos_sb[:], in_=lpos_d.ap())
            ident = cst.tile([P, P], fp32)
            nc.sync.dma_start(out=ident[:], in_=ident_d.ap())
            magic_t = cst.tile([P, 1], i32)
            nc.vector.memset(magic_t[:], RSQRT_MAGIC)

            def layer_norm_g(h3d, gname, g_bc_ap, bt_bc_ap, pool, G, D):
                # grouped LN over the last dim of h3d ([P, G, D]); in-place.
                # rstd via bit-hack + 2 Newton steps (sign self-corrects over
                # even iteration counts) -- keeps ScalarE's table on gelu/tanh.
                stats = pool.tile([P, G, 6], fp32, tag=f"lnst{gname}")
                for g in range(G):
                    nc.vector.bn_stats(out=stats[:, g, :], in_=h3d[:, g, :])
                mv = pool.tile([P, G, 2], fp32, tag=f"lnmv{gname}")
                for g in range(G):
                    nc.vector.bn_aggr(out=mv[:, g, :], in_=stats[:, g, :])
                ve = pool.tile([P, G], fp32, tag=f"lnve{gname}")
                nc.vector.tensor_scalar(out=ve[:], in0=mv[:, :, 1], scalar1=EPS,
                                        scalar2=None, op0=OP.add)  # var+eps
                yi = pool.tile([P, G], i32, tag=f"lnyi{gname}")
                nc.vector.tensor_scalar(out=yi[:], in0=ve[:].bitcast(i32), scalar1=1,
                                        scalar2=None, op0=OP.arith_shift_right)
                nc.vector.tensor_tensor(out=yi[:], in0=magic_t[:].to_broadcast([P, G]),
                                        in1=yi[:], op=OP.subtract)
                y = yi[:].bitcast(fp32)
                a = pool.tile([P, G], fp32, tag=f"lna{gname}")
                for _ in range(2):
                    nc.vector.tensor_tensor(out=a[:], in0=y, in1=y, op=OP.mult)
                    nc.vector.scalar_tensor_tensor(out=a[:], in0=a[:], scalar=1.0,
                                                   in1=ve[:], op0=OP.mult, op1=OP.mult)
                    nc.vector.tensor_scalar(out=a[:], in0=a[:], scalar1=0.5, scalar2=1.5,
                                            op0=OP.mult, op1=OP.subtract)
                    nc.vector.tensor_tensor(out=y, in0=y, in1=a[:], op=OP.mult)
                mean_bc = fbcast(mv[:, :, 0], D)
                nc.vector.tensor_tensor(out=h3d, in0=h3d, in1=mean_bc, op=OP.subtract)
                nc.vector.tensor_tensor(out=h3d, in0=h3d, in1=fbcast(y, D), op=OP.mult)
                nc.vector.tensor_tensor(out=h3d, in0=h3d, in1=g_bc_ap, op=OP.mult)
                nc.vector.tensor_tensor(out=h3d, in0=h3d, in1=bt_bc_ap, op=OP.add)

            def gbcast(ap2d, G):
                # [P, D] -> [P, G, D] broadcast over a new middle dim
                return bass.AP(tensor=ap2d.tensor, offset=ap2d.offset,
                               ap=[ap2d.ap[0], [0, G]] + list(ap2d.ap[1:]))

            pending = []

            def do_lerp(item):
                # lerp: res = w0*val0 + w1*val1, per-n with [P,1] scalars
                # (per-partition-scalar ops run ~20x faster than bcast-AP ops);
                # deferred one batch so the next network fills the DVE queue
                # while this batch's gathers stream in.
                pb, lc, gat, w0a, w1a = item
                res_f = lpool.tile([P, N, C], fp32, tag="res_f")
                for n_i in range(N):
                    w1c = w1a[:, lc * N + n_i: lc * N + n_i + 1]
                    nc.scalar.mul(out=gat[:, n_i, C:ELEM], in_=gat[:, n_i, C:ELEM], mul=w1c)
                for n_i in range(N):
                    w0c = w0a[:, lc * N + n_i: lc * N + n_i + 1]
                    nc.vector.scalar_tensor_tensor(out=res_f[:, n_i, :], in0=gat[:, n_i, 0:C],
                                                   scalar=w0c, in1=gat[:, n_i, C:ELEM],
                                                   op0=OP.mult, op1=OP.add)
                nc.sync.dma_start(
                    out=out_d.ap()[pb * L + lc * P: pb * L + (lc + 1) * P, :],
                    in_=res_f[:].rearrange("p n c -> p (n c)"))

            for b in range(BPC):
                new_pending = []
                # ======== stage A: sample_layer for batch b ========
                q_sb = qpool.tile([P, KQ, C], fp32)
                nc.sync.dma_start(out=q_sb[:], in_=query_d.ap()[b].rearrange("(k p) c -> p k c", p=P))

                h1_all = apool.tile([P, 2, J], fp32, tag="h1_all")
                for cc in range(2):
                    ph1 = ps_a.tile([P, J], fp32)
                    for k in range(KQ):
                        nc.tensor.matmul(ph1[:], lhsT=q_sb[:, k, cc * P:(cc + 1) * P],
                                         rhs=w1_sb[:, k, :], start=(k == 0), stop=(k == KQ - 1))
                    nc.vector.tensor_tensor(out=h1_all[:, cc, :], in0=ph1[:], in1=b1_bc[:], op=OP.add)
                layer_norm_g(h1_all[:], "a", gbcast(g1_bc[:], 2), gbcast(bt1_bc[:], 2), apool, 2, J)
                gel1 = apool.tile([P, 2, J], fp32, tag="gel1")
                nc.scalar.activation(gel1[:], h1_all[:], AF.Gelu)

                gelT = apool.tile([P, C], fp32)   # [j, c] transposed gelu output
                for cc in range(2):
                    pt = ps_t.tile([P, P], fp32, tag="pt")
                    nc.tensor.transpose(pt[:], gel1[:, cc, :], ident[:])
                    nc.scalar.copy(out=gelT[:, cc * P:(cc + 1) * P], in_=pt[:])

                h2_sb = h2pool.tile([P, 2, L], fp32)  # [c_part, ck, l]
                for cc2 in range(2):
                    ph2 = ps_h2.tile([P, L], fp32)
                    nc.tensor.matmul(ph2[:], lhsT=gelT[:, cc2 * P:(cc2 + 1) * P],
                                     rhs=w2_sb[:], start=True, stop=True)
                    nc.vector.tensor_tensor(out=h2_sb[:, cc2, :], in0=ph2[:], in1=b2_bc[:], op=OP.add)

                # ======== stage B, batched across the 4 l-chunks ========
                h3_all = bpool.tile([P, LC, C], fp32, tag="h3_all")
                for lc in range(LC):
                    ph3 = ps_h3.tile([P, C], fp32)
                    for ck in range(2):
                        nc.tensor.matmul(ph3[:], lhsT=h2_sb[:, ck, lc * P:(lc + 1) * P],
                                         rhs=w3_sb[:, ck, :], start=(ck == 0), stop=(ck == 1))
                    nc.vector.tensor_tensor(out=h3_all[:, lc, :], in0=ph3[:], in1=b3_bc[:], op=OP.add)
                layer_norm_g(h3_all[:], "b", gbcast(g3_bc[:], LC), gbcast(bt3_bc[:], LC), bpool, LC, C)
                gel3 = bpool.tile([P, LC, C], fp32, tag="gel3")
                nc.scalar.activation(gel3[:], h3_all[:], AF.Gelu)

                gel3T = bpool.tile([P, LC, 2, P], fp32, tag="gel3T")
                for lc in range(LC):
                    for ck2 in range(2):
                        pt3 = ps_t.tile([P, P], fp32, tag="pt")
                        nc.tensor.transpose(pt3[:], gel3[:, lc, ck2 * P:(ck2 + 1) * P], ident[:])
                        nc.scalar.copy(out=gel3T[:, lc, ck2, :], in_=pt3[:])

                padj = ps_adj.tile([P, LC * 2 * N], fp32)
                for lc in range(LC):
                    for ck2 in range(2):
                        nc.tensor.matmul(padj[:, lc * 2 * N:(lc + 1) * 2 * N],
                                         lhsT=gel3T[:, lc, ck2, :], rhs=w4_sb[:, ck2, :],
                                         start=(ck2 == 0), stop=(ck2 == 1))
                adj = bpool.tile([P, LC, 2 * N], fp32, tag="adj")
                nc.vector.tensor_tensor(out=adj[:], in0=padj[:].rearrange("p (lc f) -> p lc f", f=2 * N),
                                        in1=gbcast(b4_bc[:], LC), op=OP.add)
                th = bpool.tile([P, LC, 2 * N], fp32, tag="th")
                nc.scalar.activation(th[:], adj[:], AF.Tanh)

                off_t = bpool.tile([P, LC, 2 * N], fp32, tag="off_t")
                nc.sync.dma_start(out=off_t[:],
                                  in_=off_d.ap()[b].rearrange("(lc p) f -> p lc f", p=P))
                arg = bpool.tile([P, LC, 2 * N], fp32, tag="arg")
                # arg = 0.02*tanh(adj) + offset + ref
                nc.vector.scalar_tensor_tensor(out=arg[:], in0=th[:], scalar=0.02,
                                               in1=off_t[:], op0=OP.mult, op1=OP.add)
                nc.vector.tensor_tensor(out=arg[:], in0=arg[:], in1=ref_sb[:], op=OP.add)
                pos_t = bpool.tile([P, LC, 2 * N], fp32, tag="pos_t")
                nc.scalar.activation(pos_t[:], arg[:], AF.Tanh)
                pos_dst = bass.AP(tensor=pos_d, offset=b * L * 2 * N,
                                  ap=[[2 * N, P], [P * 2 * N, LC], [1, 2 * N]])
                nc.sync.dma_start(out=pos_dst, in_=pos_t[:])

                # pix / taps / weights, batched [P, LC*N]
                pix = bpool.tile([P, LC, N], fp32, tag="pix")
                nc.vector.tensor_scalar(out=pix[:], in0=pos_t[:, :, 1::2],
                                        scalar1=float((L - 1) / 2), scalar2=float((L - 1) / 2),
                                        op0=OP.mult, op1=OP.add)
                pixf = pix[:].rearrange("p lc n -> p (lc n)")
                # floor(pix), robust to the dtype-conversion rounding mode:
                ci = bpool.tile([P, LC * N], i32, tag="ci")
                nc.vector.tensor_copy(out=ci[:], in_=pixf)
                cf = bpool.tile([P, LC * N], fp32, tag="cf")
                nc.vector.tensor_copy(out=cf[:], in_=ci[:])
                mgt = bpool.tile([P, LC * N], fp32, tag="mgt")
                nc.vector.tensor_tensor(out=mgt[:], in0=cf[:], in1=pixf, op=OP.is_gt)
                i0f = bpool.tile([P, LC * N], fp32, tag="i0f")
                nc.vector.tensor_tensor(out=i0f[:], in0=cf[:], in1=mgt[:], op=OP.subtract)
                frac = bpool.tile([P, LC * N], fp32, tag="frac")
                nc.vector.tensor_tensor(out=frac[:], in0=pixf, in1=i0f[:], op=OP.subtract)
                nm = bpool.tile([P, LC * N], fp32, tag="nm")
                nc.vector.tensor_scalar(out=nm[:], in0=frac[:], scalar1=-1.0, scalar2=1.0,
                                        op0=OP.mult, op1=OP.add)
                m0 = bpool.tile([P, LC * N], fp32, tag="m0")
                nc.vector.tensor_tensor(out=m0[:], in0=i0f[:], in1=lpos_sb[:, 0:LC * N], op=OP.is_le)
                w0_all = bpool.tile([P, LC * N], fp32, tag="w0_all")
                nc.vector.tensor_tensor(out=w0_all[:], in0=nm[:], in1=m0[:], op=OP.mult)
                m1 = bpool.tile([P, LC * N], fp32, tag="m1")
                nc.vector.tensor_tensor(out=m1[:], in0=i0f[:], in1=lpos_sb[:, LC * N:], op=OP.is_le)
                w1_all = bpool.tile([P, LC * N], fp32, tag="w1_all")
                nc.vector.tensor_tensor(out=w1_all[:], in0=frac[:], in1=m1[:], op=OP.mult)

                # int16 flat row index = b*L + i0
                idxf = bpool.tile([P, LC * N], fp32, tag="idxf")
                nc.vector.tensor_scalar(out=idxf[:], in0=i0f[:], scalar1=float(b * L),
                                        scalar2=None, op0=OP.add)
                idx16 = bpool.tile([P, LC * N], i16, tag="idx16")
                nc.vector.tensor_copy(out=idx16[:], in_=idxf[:])

                # ---- idx wrap fold via DRAM bounce (32B descriptors) ----
                # bounce holds (l, lc, n); M[16g+q, lc*128 + j*16 + n] = idx(l=j*16+q, lc, n);
                # one DVE copy swaps free (j,n)->(n,j) per lc giving the wrapped
                # table T[q, n*8+j] = idx(i=(n*8+j)*16+q) for slot i = n*128 + l.
                dbounce = drampool.tile([P, LC * N], i16)
                nc.sync.dma_start(out=dbounce[:], in_=idx16[:])
                M_t = bpool.tile([P, LC * P], i16, tag="M_t")
                dfl = dbounce[:]
                src = bass.AP(tensor=dfl.tensor, offset=dfl.offset,
                              ap=[[LC * N, 16], [N, LC], [16 * LC * N, 8], [1, N]])
                for g in range(8):
                    dst = M_t[g * 16:(g + 1) * 16, :].rearrange(
                        "q (lc j n) -> q lc j n", j=8, n=N)
                    nc.sync.dma_start(out=dst, in_=src)
                T_all = bpool.tile([P, LC * P], i16, tag="T_all")
                nc.vector.tensor_copy(
                    out=T_all[:].rearrange("p (lc n j) -> p lc n j", j=8, n=N),
                    in_=M_t[:].rearrange("p (lc j n) -> p lc n j", j=8, n=N))

                for lc in range(LC):
                    gat = gpool.tile([P, N, ELEM], bf16)
                    nc.gpsimd.dma_gather(
                        out_ap=gat[:], in_ap=x_win, idxs_ap=T_all[:, lc * P:(lc + 1) * P],
                        num_idxs=NIDX, num_idxs_reg=NIDX,
                        elem_size=ELEM, elem_step=C, single_packet=False)
                    new_pending.append((b, lc, gat, w0_all, w1_all))
                    if pending:
                        do_lerp(pending.pop(0))
                while pending:
                    do_lerp(pending.pop(0))
                pending = new_pending
            while pending:
                do_lerp(pending.pop(0))

    nc.compile()
    return nc


def _consts():
    i = np.arange(L, dtype=np.float32)
    n = np.arange(N, dtype=np.float32) + 0.5
    ref_y = n[None, :] * 2.0 * (i[:, None] + 1e-9) / (N * L) - 1.0   # [L, N]
    ref = np.zeros((L, 2 * N), np.float32)
    ref[:, 1::2] = ref_y
    lpos = np.zeros((P, 2 * LC * N), np.float32)
    for lc in range(LC):
        col = (lc * P + np.arange(P, dtype=np.float32))[:, None]
        lpos[:, lc * N:(lc + 1) * N] = col
        lpos[:, LC * N + lc * N: LC * N + (lc + 1) * N] = col - 1.0
    ident = np.eye(P, dtype=np.float32)
    return ref, lpos, ident


def kernel(query, x, offset, w1, b1, g1, bt1, w2, b2, w3, b3, g3, bt3, w4, b4):
    from concourse.bass_utils import run_bass_kernel_spmd

    query = np.ascontiguousarray(np.asarray(query, dtype=np.float32))
    x = np.ascontiguousarray(np.asarray(x, dtype=np.float32))
    offset = np.ascontiguousarray(np.asarray(offset, dtype=np.float32))
    params = {k: np.ascontiguousarray(np.asarray(v, dtype=np.float32)) for k, v in
              dict(w1=w1, b1=b1, g1=g1, bt1=bt1, w2=w2, b2=b2, w3=w3, b3=b3,
                   g3=g3, bt3=bt3, w4=w4, b4=b4).items()}

    import ml_dtypes
    x16 = x.astype(ml_dtypes.bfloat16)

    if "nc" not in _CACHE:
        _CACHE["nc"] = _build()
    nc = _CACHE["nc"]

    ref, lpos, ident = _consts()
    in_maps = []
    for c in range(NCORES):
        bs = slice(c * BPC, (c + 1) * BPC)
        m = dict(params)
        m["query"] = query[bs]
        m["x16"] = x16[bs].reshape(BPC * L, C)
        m["offset"] = offset[bs]
        m["cst_ref"] = ref
        m["cst_lpos"] = lpos
        m["cst_ident"] = ident
        in_maps.append(m)

    trace = os.environ.get("KERNEL_TRACE", "0") == "1"
    res = run_bass_kernel_spmd(nc, in_maps, core_ids=list(range(NCORES)), trace=trace)
    _CACHE["last_results"] = res

    out = np.concatenate([r["out"].reshape(BPC, L, N, C) for r in res.results], axis=0)
    pos = np.concatenate([r["pos"].reshape(BPC, L, N, 2) for r in res.results], axis=0)
    return out, offset, pos


# revision 27
# speedup vs baseline: 1.0335x; 1.0335x over previous
"""Trainium2 Bass kernel for nn_AdjustNet (sparse_attention).

Data-parallel over batch B=32 across 8 NeuronCores (4 batch elements per core).
Per batch element (all on-device):
  sample_layer : gelu(ln(query^T @ w1 + b1)) @ w2 + b2      (over sequence axis)
  adjust_layer : adj = gelu(ln(q @ w3 + b3)) @ w4 + b4 ; tanh*0.02
  pos = tanh(offset + adj + ref)
  out = causal bilinear gather of x rows (dma_gather, bf16) + lerp.

Self-contained: hardcodes shapes; host-side work is only sharding/assembly and
constant-table generation.
"""
import os
import numpy as np

B, L, C, N = 32, 512, 256, 16
NCORES = 8
BPC = B // NCORES          # batch elements per core
P = 128
KQ = L // P                # 4 contraction chunks over sequence (mm1)
LC = L // P                # 4 l-chunks
J = L // 4                 # 128 bottleneck dim of sample_layer
EPS = 1e-12
ELEM = 2 * C               # gather element: two consecutive x rows
NIDX = P * N               # gather indices per (batch, l-chunk)
RSQRT_MAGIC = 0x5F3759DF

_CACHE = {}


def _build():
    import concourse.bacc as bacc
    import concourse.bass as bass
    import concourse.mybir as mybir
    import concourse.tile as tile
    from concourse.library_config import mlp

    fp32 = mybir.dt.float32
    bf16 = mybir.dt.bfloat16
    i16 = mybir.dt.int16
    i32 = mybir.dt.int32
    AF = mybir.ActivationFunctionType
    OP = mybir.AluOpType

    nc = bacc.Bacc("TRN2", target_bir_lowering=False, debug=False)

    # ---------------- per-core DRAM I/O ----------------
    query_d = nc.dram_tensor("query", [BPC, L, C], fp32, kind="ExternalInput")
    off_d = nc.dram_tensor("offset", [BPC, L, 2 * N], fp32, kind="ExternalInput")
    w1_d = nc.dram_tensor("w1", [L, J], fp32, kind="ExternalInput")
    b1_d = nc.dram_tensor("b1", [J], fp32, kind="ExternalInput")
    g1_d = nc.dram_tensor("g1", [J], fp32, kind="ExternalInput")
    bt1_d = nc.dram_tensor("bt1", [J], fp32, kind="ExternalInput")
    w2_d = nc.dram_tensor("w2", [J, L], fp32, kind="ExternalInput")
    b2_d = nc.dram_tensor("b2", [L], fp32, kind="ExternalInput")
    w3_d = nc.dram_tensor("w3", [C, C], fp32, kind="ExternalInput")
    b3_d = nc.dram_tensor("b3", [C], fp32, kind="ExternalInput")
    g3_d = nc.dram_tensor("g3", [C], fp32, kind="ExternalInput")
    bt3_d = nc.dram_tensor("bt3", [C], fp32, kind="ExternalInput")
    w4_d = nc.dram_tensor("w4", [C, 2 * N], fp32, kind="ExternalInput")
    b4_d = nc.dram_tensor("b4", [2 * N], fp32, kind="ExternalInput")
    ref_d = nc.dram_tensor("cst_ref", [L, 2 * N], fp32, kind="ExternalInput")
    lpos_d = nc.dram_tensor("cst_lpos", [P, 2 * LC * N], fp32, kind="ExternalInput")
    ident_d = nc.dram_tensor("cst_ident", [P, P], fp32, kind="ExternalInput")

    out_d = nc.dram_tensor("out", [BPC * L, N * C], fp32, kind="ExternalOutput")
    pos_d = nc.dram_tensor("pos", [BPC * L, 2 * N], fp32, kind="ExternalOutput")

    x16_d = nc.dram_tensor("x16", [BPC * L, C], bf16, kind="ExternalInput")

    def pbcast(dram_1d, n):
        # replicate a [n] DRAM vector across 128 partitions
        return bass.AP(tensor=dram_1d, offset=0, ap=[[0, P], [1, n]])

    def fbcast(ap2d, inner):
        # broadcast a [P, F] AP along a new innermost dim of size `inner`
        return bass.AP(tensor=ap2d.tensor, offset=ap2d.offset, ap=list(ap2d.ap) + [[0, inner]])

    # overlapping 2-row window of bf16 x: idx i -> 512 consecutive bf16 from row i
    x_win = bass.AP(tensor=x16_d, offset=0, ap=[[C, BPC * L - 1], [1, ELEM]])

    with tile.TileContext(nc) as tc:
        import contextlib
        with contextlib.ExitStack() as ctx:
            cst = ctx.enter_context(tc.tile_pool(name="cst", bufs=1))
            qpool = ctx.enter_context(tc.tile_pool(name="qpool", bufs=2))
            apool = ctx.enter_context(tc.tile_pool(name="apool", bufs=2))
            h2pool = ctx.enter_context(tc.tile_pool(name="h2pool", bufs=2))
            bpool = ctx.enter_context(tc.tile_pool(name="bpool", bufs=2))
            gpool = ctx.enter_context(tc.tile_pool(name="gpool", bufs=5))
            lpool = ctx.enter_context(tc.tile_pool(name="lpool", bufs=2))
            drampool = ctx.enter_context(tc.tile_pool(name="drampool", bufs=3, space="DRAM"))
            ps_a = ctx.enter_context(tc.tile_pool(name="ps_a", bufs=2, space="PSUM"))
            ps_t = ctx.enter_context(tc.tile_pool(name="ps_t", bufs=2, space="PSUM"))
            ps_h2 = ctx.enter_context(tc.tile_pool(name="ps_h2", bufs=1, space="PSUM"))
            ps_h3 = ctx.enter_context(tc.tile_pool(name="ps_h3", bufs=2, space="PSUM"))
            ps_adj = ctx.enter_context(tc.tile_pool(name="ps_adj", bufs=1, space="PSUM"))

            nc.gpsimd.load_library(mlp)

            # ---------------- constants into SBUF ----------------
            w1_sb = cst.tile([P, KQ, J], fp32)
            nc.sync.dma_start(out=w1_sb[:], in_=w1_d.ap().rearrange("(k p) j -> p k j", p=P))
            w2_sb = cst.tile([P, L], fp32)
            nc.sync.dma_start(out=w2_sb[:], in_=w2_d.ap())
            w3_sb = cst.tile([P, 2, C], fp32)
            nc.sync.dma_start(out=w3_sb[:], in_=w3_d.ap().rearrange("(k p) c -> p k c", p=P))
            w4_sb = cst.tile([P, 2, 2 * N], fp32)
            nc.sync.dma_start(out=w4_sb[:], in_=w4_d.ap().rearrange("(k p) m -> p k m", p=P))
            b1_bc = cst.tile([P, J], fp32)
            nc.sync.dma_start(out=b1_bc[:], in_=pbcast(b1_d, J))
            g1_bc = cst.tile([P, J], fp32)
            nc.sync.dma_start(out=g1_bc[:], in_=pbcast(g1_d, J))
            bt1_bc = cst.tile([P, J], fp32)
            nc.sync.dma_start(out=bt1_bc[:], in_=pbcast(bt1_d, J))
            b2_bc = cst.tile([P, L], fp32)
            nc.sync.dma_start(out=b2_bc[:], in_=pbcast(b2_d, L))
            b3_bc = cst.tile([P, C], fp32)
            nc.sync.dma_start(out=b3_bc[:], in_=pbcast(b3_d, C))
            g3_bc = cst.tile([P, C], fp32)
            nc.sync.dma_start(out=g3_bc[:], in_=pbcast(g3_d, C))
            bt3_bc = cst.tile([P, C], fp32)
            nc.sync.dma_start(out=bt3_bc[:], in_=pbcast(bt3_d, C))
            b4_bc = cst.tile([P, 2 * N], fp32)
            nc.sync.dma_start(out=b4_bc[:], in_=pbcast(b4_d, 2 * N))
            ref_sb = cst.tile([P, LC, 2 * N], fp32)
            nc.sync.dma_start(out=ref_sb[:], in_=ref_d.ap().rearrange("(lc p) f -> p lc f", p=P))
            lpos_sb = cst.tile([P, 2 * LC * N], fp32)
            nc.sync.dma_start(out=lpos_sb[:], in_=lpos_d.ap())
            ident = cst.tile([P, P], fp32)
            nc.sync.dma_start(out=ident[:], in_=ident_d.ap())
            magic_t = cst.tile([P, 1], i32)
            nc.vector.memset(magic_t[:], RSQRT_MAGIC)

            def layer_norm_g(h3d, gname, g_bc_ap, bt_bc_ap, pool, G, D):
                # grouped LN over the last dim of h3d ([P, G, D]); in-place.
                # rstd via bit-hack + 2 Newton steps (sign self-corrects over
                # even iteration counts) -- keeps ScalarE's table on gelu/tanh.
                stats = pool.tile([P, G, 6], fp32, tag=f"lnst{gname}")
                for g in range(G):
                    nc.vector.bn_stats(out=stats[:, g, :], in_=h3d[:, g, :])
                mv = pool.tile([P, G, 2], fp32, tag=f"lnmv{gname}")
                for g in range(G):
                    nc.vector.bn_aggr(out=mv[:, g, :], in_=stats[:, g, :])
                ve = pool.tile([P, G], fp32, tag=f"lnve{gname}")
                nc.vector.tensor_scalar(out=ve[:], in0=mv[:, :, 1], scalar1=EPS,
                                        scalar2=None, op0=OP.add)  # var+eps
                yi = pool.tile([P, G], i32, tag=f"lnyi{gname}")
                nc.vector.tensor_scalar(out=yi[:], in0=ve[:].bitcast(i32), scalar1=1,
                                        scalar2=None, op0=OP.arith_shift_right)
                nc.vector.tensor_tensor(out=yi[:], in0=magic_t[:].to_broadcast([P, G]),
                                        in1=yi[:], op=OP.subtract)
                y = yi[:].bitcast(fp32)
                a = pool.tile([P, G], fp32, tag=f"lna{gname}")
                for _ in range(2):
                    nc.vector.tensor_tensor(out=a[:], in0=y, in1=y, op=OP.mult)
                    nc.vector.scalar_tensor_tensor(out=a[:], in0=a[:], scalar=1.0,
                                                   in1=ve[:], op0=OP.mult, op1=OP.mult)
                    nc.vector.tensor_scalar(out=a[:], in0=a[:], scalar1=0.5, scalar2=1.5,
                                            op0=OP.mult, op1=OP.subtract)
                    nc.vector.tensor_tensor(out=y, in0=y, in1=a[:], op=OP.mult)
                mean_bc = fbcast(mv[:, :, 0], D)
                nc.vector.tensor_tensor(out=h3d, in0=h3d, in1=mean_bc, op=OP.subtract)
                nc.vector.tensor_tensor(out=h3d, in0=h3d, in1=fbcast(y, D), op=OP.mult)
                nc.vector.tensor_tensor(out=h3d, in0=h3d, in1=g_bc_ap, op=OP.mult)
                nc.vector.tensor_tensor(out=h3d, in0=h3d, in1=bt_bc_ap, op=OP.add)

            def gbcast(ap2d, G):
                # [P, D] -> [P, G, D] broadcast over a new middle dim
                return bass.AP(tensor=ap2d.tensor, offset=ap2d.offset,
                               ap=[ap2d.ap[0], [0, G]] + list(ap2d.ap[1:]))

            pending = []

            def do_lerp(item):
                # lerp: res = w0*val0 + w1*val1, per-n with [P,1] scalars
                # (per-partition-scalar ops run ~20x faster than bcast-AP ops);
                # deferred one batch so the next network fills the DVE queue
                # while this batch's gathers stream in.
                pb, lc, gat, w0a, w1a = item
                res = lpool.tile([P, N, C], bf16, tag="res")
                for n_i in range(N):
                    w1c = w1a[:, lc * N + n_i: lc * N + n_i + 1]
                    nc.scalar.mul(out=gat[:, n_i, C:ELEM], in_=gat[:, n_i, C:ELEM], mul=w1c)
                for n_i in range(N):
                    w0c = w0a[:, lc * N + n_i: lc * N + n_i + 1]
                    nc.vector.scalar_tensor_tensor(out=res[:, n_i, :], in0=gat[:, n_i, 0:C],
                                                   scalar=w0c, in1=gat[:, n_i, C:ELEM],
                                                   op0=OP.mult, op1=OP.add)
                # SWDGE cast-DMA widens bf16 -> fp32 on the way out
                nc.gpsimd.dma_start(
                    out=out_d.ap()[pb * L + lc * P: pb * L + (lc + 1) * P, :],
                    in_=res[:].rearrange("p n c -> p (n c)"))

            for b in range(BPC):
                new_pending = []
                # ======== stage A: sample_layer for batch b ========
                q_sb = qpool.tile([P, KQ, C], fp32)
                nc.sync.dma_start(out=q_sb[:], in_=query_d.ap()[b].rearrange("(k p) c -> p k c", p=P))

                h1_all = apool.tile([P, 2, J], fp32, tag="h1_all")
                for cc in range(2):
                    ph1 = ps_a.tile([P, J], fp32)
                    for k in range(KQ):
                        nc.tensor.matmul(ph1[:], lhsT=q_sb[:, k, cc * P:(cc + 1) * P],
                                         rhs=w1_sb[:, k, :], start=(k == 0), stop=(k == KQ - 1))
                    nc.vector.tensor_tensor(out=h1_all[:, cc, :], in0=ph1[:], in1=b1_bc[:], op=OP.add)
                layer_norm_g(h1_all[:], "a", gbcast(g1_bc[:], 2), gbcast(bt1_bc[:], 2), apool, 2, J)
                gel1 = apool.tile([P, 2, J], fp32, tag="gel1")
                nc.scalar.activation(gel1[:], h1_all[:], AF.Gelu)

                gelT = apool.tile([P, C], fp32)   # [j, c] transposed gelu output
                for cc in range(2):
                    pt = ps_t.tile([P, P], fp32, tag="pt")
                    nc.tensor.transpose(pt[:], gel1[:, cc, :], ident[:])
                    nc.scalar.copy(out=gelT[:, cc * P:(cc + 1) * P], in_=pt[:])

                h2_sb = h2pool.tile([P, 2, L], fp32)  # [c_part, ck, l]
                for cc2 in range(2):
                    ph2 = ps_h2.tile([P, L], fp32)
                    nc.tensor.matmul(ph2[:], lhsT=gelT[:, cc2 * P:(cc2 + 1) * P],
                                     rhs=w2_sb[:], start=True, stop=True)
                    nc.vector.tensor_tensor(out=h2_sb[:, cc2, :], in0=ph2[:], in1=b2_bc[:], op=OP.add)

                # ======== stage B, batched across the 4 l-chunks ========
                h3_all = bpool.tile([P, LC, C], fp32, tag="h3_all")
                for lc in range(LC):
                    ph3 = ps_h3.tile([P, C], fp32)
                    for ck in range(2):
                        nc.tensor.matmul(ph3[:], lhsT=h2_sb[:, ck, lc * P:(lc + 1) * P],
                                         rhs=w3_sb[:, ck, :], start=(ck == 0), stop=(ck == 1))
                    nc.vector.tensor_tensor(out=h3_all[:, lc, :], in0=ph3[:], in1=b3_bc[:], op=OP.add)
                layer_norm_g(h3_all[:], "b", gbcast(g3_bc[:], LC), gbcast(bt3_bc[:], LC), bpool, LC, C)
                gel3 = bpool.tile([P, LC, C], fp32, tag="gel3")
                nc.scalar.activation(gel3[:], h3_all[:], AF.Gelu)

                gel3T = bpool.tile([P, LC, 2, P], fp32, tag="gel3T")
                for lc in range(LC):
                    for ck2 in range(2):
                        pt3 = ps_t.tile([P, P], fp32, tag="pt")
                        nc.tensor.transpose(pt3[:], gel3[:, lc, ck2 * P:(ck2 + 1) * P], ident[:])
                        nc.scalar.copy(out=gel3T[:, lc, ck2, :], in_=pt3[:])

                padj = ps_adj.tile([P, LC * 2 * N], fp32)
                for lc in range(LC):
                    for ck2 in range(2):
                        nc.tensor.matmul(padj[:, lc * 2 * N:(lc + 1) * 2 * N],
                                         lhsT=gel3T[:, lc, ck2, :], rhs=w4_sb[:, ck2, :],
                                         start=(ck2 == 0), stop=(ck2 == 1))
                adj = bpool.tile([P, LC, 2 * N], fp32, tag="adj")
                nc.vector.tensor_tensor(out=adj[:], in0=padj[:].rearrange("p (lc f) -> p lc f", f=2 * N),
                                        in1=gbcast(b4_bc[:], LC), op=OP.add)
                th = bpool.tile([P, LC, 2 * N], fp32, tag="th")
                nc.scalar.activation(th[:], adj[:], AF.Tanh)

                off_t = bpool.tile([P, LC, 2 * N], fp32, tag="off_t")
                nc.sync.dma_start(out=off_t[:],
                                  in_=off_d.ap()[b].rearrange("(lc p) f -> p lc f", p=P))
                arg = bpool.tile([P, LC, 2 * N], fp32, tag="arg")
                # arg = 0.02*tanh(adj) + offset + ref
                nc.vector.scalar_tensor_tensor(out=arg[:], in0=th[:], scalar=0.02,
                                               in1=off_t[:], op0=OP.mult, op1=OP.add)
                nc.vector.tensor_tensor(out=arg[:], in0=arg[:], in1=ref_sb[:], op=OP.add)
                pos_t = bpool.tile([P, LC, 2 * N], fp32, tag="pos_t")
                nc.scalar.activation(pos_t[:], arg[:], AF.Tanh)
                pos_dst = bass.AP(tensor=pos_d, offset=b * L * 2 * N,
                                  ap=[[2 * N, P], [P * 2 * N, LC], [1, 2 * N]])
                nc.sync.dma_start(out=pos_dst, in_=pos_t[:])

                # pix / taps / weights, batched [P, LC*N]
                pix = bpool.tile([P, LC, N], fp32, tag="pix")
                nc.vector.tensor_scalar(out=pix[:], in0=pos_t[:, :, 1::2],
                                        scalar1=float((L - 1) / 2), scalar2=float((L - 1) / 2),
                                        op0=OP.mult, op1=OP.add)
                pixf = pix[:].rearrange("p lc n -> p (lc n)")
                # floor(pix), robust to the dtype-conversion rounding mode:
                ci = bpool.tile([P, LC * N], i32, tag="ci")
                nc.vector.tensor_copy(out=ci[:], in_=pixf)
                cf = bpool.tile([P, LC * N], fp32, tag="cf")
                nc.vector.tensor_copy(out=cf[:], in_=ci[:])
                mgt = bpool.tile([P, LC * N], fp32, tag="mgt")
                nc.vector.tensor_tensor(out=mgt[:], in0=cf[:], in1=pixf, op=OP.is_gt)
                i0f = bpool.tile([P, LC * N], fp32, tag="i0f")
                nc.vector.tensor_tensor(out=i0f[:], in0=cf[:], in1=mgt[:], op=OP.subtract)
                frac = bpool.tile([P, LC * N], fp32, tag="frac")
                nc.vector.tensor_tensor(out=frac[:], in0=pixf, in1=i0f[:], op=OP.subtract)
                nm = bpool.tile([P, LC * N], fp32, tag="nm")
                nc.vector.tensor_scalar(out=nm[:], in0=frac[:], scalar1=-1.0, scalar2=1.0,
                                        op0=OP.mult, op1=OP.add)
                m0 = bpool.tile([P, LC * N], fp32, tag="m0")
                nc.vector.tensor_tensor(out=m0[:], in0=i0f[:], in1=lpos_sb[:, 0:LC * N], op=OP.is_le)
                w0_all = bpool.tile([P, LC * N], fp32, tag="w0_all")
                nc.vector.tensor_tensor(out=w0_all[:], in0=nm[:], in1=m0[:], op=OP.mult)
                m1 = bpool.tile([P, LC * N], fp32, tag="m1")
                nc.vector.tensor_tensor(out=m1[:], in0=i0f[:], in1=lpos_sb[:, LC * N:], op=OP.is_le)
                w1_all = bpool.tile([P, LC * N], fp32, tag="w1_all")
                nc.vector.tensor_tensor(out=w1_all[:], in0=frac[:], in1=m1[:], op=OP.mult)

                # int16 flat row index = b*L + i0
                idxf = bpool.tile([P, LC * N], fp32, tag="idxf")
                nc.vector.tensor_scalar(out=idxf[:], in0=i0f[:], scalar1=float(b * L),
                                        scalar2=None, op0=OP.add)
                idx16 = bpool.tile([P, LC * N], i16, tag="idx16")
                nc.vector.tensor_copy(out=idx16[:], in_=idxf[:])

                # ---- idx wrap fold via DRAM bounce (32B descriptors) ----
                # bounce holds (l, lc, n); M[16g+q, lc*128 + j*16 + n] = idx(l=j*16+q, lc, n);
                # one DVE copy swaps free (j,n)->(n,j) per lc giving the wrapped
                # table T[q, n*8+j] = idx(i=(n*8+j)*16+q) for slot i = n*128 + l.
                dbounce = drampool.tile([P, LC * N], i16)
                nc.sync.dma_start(out=dbounce[:], in_=idx16[:])
                M_t = bpool.tile([P, LC * P], i16, tag="M_t")
                dfl = dbounce[:]
                src = bass.AP(tensor=dfl.tensor, offset=dfl.offset,
                              ap=[[LC * N, 16], [N, LC], [16 * LC * N, 8], [1, N]])
                for g in range(8):
                    dst = M_t[g * 16:(g + 1) * 16, :].rearrange(
                        "q (lc j n) -> q lc j n", j=8, n=N)
                    nc.sync.dma_start(out=dst, in_=src)
                T_all = bpool.tile([P, LC * P], i16, tag="T_all")
                nc.vector.tensor_copy(
                    out=T_all[:].rearrange("p (lc n j) -> p lc n j", j=8, n=N),
                    in_=M_t[:].rearrange("p (lc j n) -> p lc n j", j=8, n=N))

                for lc in range(LC):
                    gat = gpool.tile([P, N, ELEM], bf16)
                    nc.gpsimd.dma_gather(
                        out_ap=gat[:], in_ap=x_win, idxs_ap=T_all[:, lc * P:(lc + 1) * P],
                        num_idxs=NIDX, num_idxs_reg=NIDX,
                        elem_size=ELEM, elem_step=C, single_packet=False)
                    new_pending.append((b, lc, gat, w0_all, w1_all))
                    if pending:
                        do_lerp(pending.pop(0))
                while pending:
                    do_lerp(pending.pop(0))
                pending = new_pending
            while pending:
                do_lerp(pending.pop(0))

    nc.compile()
    return nc


def _consts():
    i = np.arange(L, dtype=np.float32)
    n = np.arange(N, dtype=np.float32) + 0.5
    ref_y = n[None, :] * 2.0 * (i[:, None] + 1e-9) / (N * L) - 1.0   # [L, N]
    ref = np.zeros((L, 2 * N), np.float32)
    ref[:, 1::2] = ref_y
    lpos = np.zeros((P, 2 * LC * N), np.float32)
    for lc in range(LC):
        col = (lc * P + np.arange(P, dtype=np.float32))[:, None]
        lpos[:, lc * N:(lc + 1) * N] = col
        lpos[:, LC * N + lc * N: LC * N + (lc + 1) * N] = col - 1.0
    ident = np.eye(P, dtype=np.float32)
    return ref, lpos, ident


def kernel(query, x, offset, w1, b1, g1, bt1, w2, b2, w3, b3, g3, bt3, w4, b4):
    from concourse.bass_utils import run_bass_kernel_spmd

    query = np.ascontiguousarray(np.asarray(query, dtype=np.float32))
    x = np.ascontiguousarray(np.asarray(x, dtype=np.float32))
    offset = np.ascontiguousarray(np.asarray(offset, dtype=np.float32))
    params = {k: np.ascontiguousarray(np.asarray(v, dtype=np.float32)) for k, v in
              dict(w1=w1, b1=b1, g1=g1, bt1=bt1, w2=w2, b2=b2, w3=w3, b3=b3,
                   g3=g3, bt3=bt3, w4=w4, b4=b4).items()}

    import ml_dtypes
    x16 = x.astype(ml_dtypes.bfloat16)

    if "nc" not in _CACHE:
        _CACHE["nc"] = _build()
    nc = _CACHE["nc"]

    ref, lpos, ident = _consts()
    in_maps = []
    for c in range(NCORES):
        bs = slice(c * BPC, (c + 1) * BPC)
        m = dict(params)
        m["query"] = query[bs]
        m["x16"] = x16[bs].reshape(BPC * L, C)
        m["offset"] = offset[bs]
        m["cst_ref"] = ref
        m["cst_lpos"] = lpos
        m["cst_ident"] = ident
        in_maps.append(m)

    trace = os.environ.get("KERNEL_TRACE", "0") == "1"
    res = run_bass_kernel_spmd(nc, in_maps, core_ids=list(range(NCORES)), trace=trace)
    _CACHE["last_results"] = res

    out = np.concatenate([r["out"].reshape(BPC, L, N, C) for r in res.results], axis=0)
    pos = np.concatenate([r["pos"].reshape(BPC, L, N, 2) for r in res.results], axis=0)
    return out, offset, pos
